# revision 16
# baseline (speedup 1.0000x reference)
"""Trainium2 Bass kernel for nn_Attr_sampler (topk_masking).

Pipeline per node: z = h@W1 (+b1); LayerNorm; ReLU; u = y@(gamma*W2); per-graph
top-k on u (monotone in sigmoid(u)); logits = sigmoid(u+b2);
x_new = mask ? x*logits : noise; edge_mask = nodes touched by edge_index.

Strategy:
- 8-way data parallel over equal 16384-node shards (128 tiles of 128 nodes).
- Matmul1 in three full-rate float32r (FP22) passes via exact hi/lo splitting:
  z = hi@Whi + lo@Whi + hi@Wlo, accuracy ~ fp32 (verified 4e-7 on HW).
- Pre-sigmoid scores AllGathered; every core bisects per-graph thresholds for
  all 256 graphs on [128, P] padded rows (static program, no dynamic offsets).
- Per-node threshold field built from host-supplied interval masks with two
  accumulated PE matmuls (telescoped staircase over sorted graph ids).
- x_new applied as x*a + noise*b with a = sigmoid(u)*mask, b = 1-mask.
- edge_mask is an independent int-only side output computed on host.
"""
import math
from contextlib import ExitStack
from dataclasses import dataclass

import numpy as np

import concourse.bass as bass
import concourse.bacc as bacc
import concourse.mybir as mybir
import concourse.tile as tile
from concourse import masks as _cmasks
from concourse.bass_utils import run_bass_kernel_spmd

F32 = mybir.dt.float32
F32R = mybir.dt.float32r
I32 = mybir.dt.int32
AF = mybir.ActivationFunctionType
ALU = mybir.AluOpType


@dataclass(frozen=True)
class Cfg:
    NC: int = 8          # cores
    S: int = 16384       # nodes per core
    HID: int = 512       # h feature dim
    ZD: int = 1024       # hidden dim (2*HID)
    XD: int = 256        # x feature dim
    NG: int = 256        # graphs
    P: int = 608         # padded slots per graph row
    NITER: int = 30      # bisection iterations
    BLO: float = -64.0   # bisection bounds for pre-sigmoid scores
    BHI: float = 64.0

    @property
    def T(self):         # node tiles per core
        return self.S // 128

    @property
    def GC(self):        # graph chunks of 128
        return (self.NG + 127) // 128

    @property
    def NTOT(self):
        return self.NC * self.S


def build_program(cfg: Cfg, b2: float):
    """One SPMD program; all per-core variation comes via input tensors."""
    c = cfg
    KCH = c.HID // 128          # contraction chunks (4)
    NH = c.ZD // 512            # 512-wide feature halves (2)
    XW = 4                      # node tiles per x-apply iteration
    nc = bacc.Bacc("TRN2", target_bir_lowering=False, debug=False,
                   num_devices=cfg.NC)

    i_h = nc.declare_dram_parameter("h_sh", [c.S, c.HID], F32, isOutput=False)
    i_x = nc.declare_dram_parameter("x_sh", [c.S, c.XD], F32, isOutput=False)
    i_n = nc.declare_dram_parameter("n_sh", [c.S, c.XD], F32, isOutput=False)
    i_w1 = nc.declare_dram_parameter("w1", [c.HID, c.ZD], F32, isOutput=False)
    i_w2 = nc.declare_dram_parameter("w2rep", [128, c.ZD], F32, isOutput=False)
    i_sel = nc.declare_dram_parameter("sel", [c.GC, 128, c.P], I32, isOutput=False)
    i_k = nc.declare_dram_parameter("kvec", [c.GC, 128, 1], F32, isOutput=False)
    i_r1 = nc.declare_dram_parameter("r1m", [c.GC, 128, c.T], F32, isOutput=False)
    i_r2 = nc.declare_dram_parameter("r2m", [c.GC, 128, c.T], F32, isOutput=False)
    i_pm = nc.declare_dram_parameter("pmm", [c.GC, 128, 128], F32, isOutput=False)
    o_lg = nc.declare_dram_parameter("logits_sh", [c.S], F32, isOutput=True)
    o_xn = nc.declare_dram_parameter("xnew_sh", [c.S, c.XD], F32, isOutput=True)

    with tile.TileContext(nc) as tc, ExitStack() as ctx:
        consts = ctx.enter_context(tc.tile_pool(name="consts", bufs=1))
        dram = ctx.enter_context(tc.tile_pool(name="dram", bufs=1, space="DRAM"))

        ident = consts.tile([128, 128], F32)
        io_r = consts.tile([128, 128], I32)
        nc.gpsimd.iota(io_r[:], pattern=[[1, 128]], base=0, channel_multiplier=0)
        io_p = consts.tile([128, 1], I32)
        nc.gpsimd.iota(io_p[:], pattern=[[1, 1]], base=0, channel_multiplier=1)
        io_pf = consts.tile([128, 1], F32)
        nc.vector.tensor_copy(io_pf[:], io_p[:])
        io_rf = consts.tile([128, 128], F32)
        nc.vector.tensor_copy(io_rf[:], io_r[:])
        nc.vector.tensor_scalar(ident[:], io_rf[:], io_pf[:], None, ALU.is_equal)
        epsb = consts.tile([128, 1], F32)
        nc.vector.memset(epsb[:], 1e-5)
        b2b = consts.tile([128, 1], F32)
        nc.vector.memset(b2b[:], float(b2))

        # ---- stage + split weights (one-time) ----
        w1f = consts.tile([128, KCH, c.ZD], F32)
        nc.sync.dma_start(w1f[:], i_w1.rearrange("(k p) m -> p k m", p=128))
        w1hi = consts.tile([128, KCH, c.ZD], F32R)
        nc.vector.tensor_copy(w1hi[:], w1f[:])
        w1lo = consts.tile([128, KCH, c.ZD], F32R)
        nc.vector.tensor_tensor(out=w1lo[:], in0=w1f[:],
                                in1=w1hi.bitcast(F32)[:], op=ALU.subtract)
        w2rep = consts.tile([128, c.ZD], F32)
        nc.sync.dma_start(w2rep[:], i_w2[:])

        upre = consts.tile([128, c.T], F32)   # pre-sigmoid scores, [p, t]

        # ---- main loop over node tiles (own stack so PSUM frees for tail) ----
        main_ctx = ExitStack()
        hpool = main_ctx.enter_context(tc.tile_pool(name="hp", bufs=3))
        spool = main_ctx.enter_context(tc.tile_pool(name="sp", bufs=2))
        tpool = main_ctx.enter_context(tc.tile_pool(name="tp", bufs=2, space="PSUM"))
        zpool = main_ctx.enter_context(tc.tile_pool(name="zp", bufs=2, space="PSUM"))
        ypool = main_ctx.enter_context(tc.tile_pool(name="yp", bufs=2))
        smallp = main_ctx.enter_context(tc.tile_pool(name="smp", bufs=3))

        h_r = i_h.rearrange("(t p) f -> t p f", p=128)
        for t in range(c.T):
            ht = hpool.tile([128, c.HID], F32, tag="ht")
            nc.sync.dma_start(ht[:], h_r[t])
            hhi = spool.tile([128, c.HID], F32R, tag="hhi")
            nc.vector.tensor_copy(hhi[:], ht[:])
            hlo = spool.tile([128, c.HID], F32R, tag="hlo")
            nc.vector.tensor_tensor(out=hlo[:], in0=ht[:],
                                    in1=hhi.bitcast(F32)[:], op=ALU.subtract)
            # transpose both (PE), PSUM -> SBUF copies on ACT
            tps = tpool.tile([128, 2 * c.HID], F32, tag="tps")
            for k in range(KCH):
                nc.tensor.transpose(tps[:, 128 * k:128 * (k + 1)],
                                    hhi.bitcast(F32)[:, 128 * k:128 * (k + 1)],
                                    ident[:])
            for k in range(KCH):
                nc.tensor.transpose(tps[:, c.HID + 128 * k:c.HID + 128 * (k + 1)],
                                    hlo.bitcast(F32)[:, 128 * k:128 * (k + 1)],
                                    ident[:])
            hiT = spool.tile([128, c.HID], F32R, tag="hiT")
            nc.scalar.copy(hiT[:], tps[:, 0:c.HID])
            loT = spool.tile([128, c.HID], F32R, tag="loT")
            nc.scalar.copy(loT[:], tps[:, c.HID:2 * c.HID])

            zps = zpool.tile([128, c.ZD], F32, tag="zps")
            for hh in range(NH):
                i = 0
                for k in range(KCH):
                    for (a, b) in ((hiT, w1hi), (loT, w1hi), (hiT, w1lo)):
                        nc.tensor.matmul(
                            zps[:, 512 * hh:512 * (hh + 1)],
                            a[:, 128 * k:128 * (k + 1)],
                            b[:, k, 512 * hh:512 * (hh + 1)],
                            start=(i == 0), stop=(i == 3 * KCH - 1))
                        i += 1

            stats = smallp.tile([128, NH, 6], F32, tag="stats")
            for hh in range(NH):
                nc.vector.bn_stats(stats[:, hh, :], zps[:, 512 * hh:512 * (hh + 1)])
            mv = smallp.tile([128, 2], F32, tag="mv")
            nc.vector.bn_aggr(mv[:], stats[:])
            # s = 1/sqrt(var+eps) with one Newton refine of the sqrt
            y0 = smallp.tile([128, 1], F32, tag="y0")
            nc.scalar.activation(y0[:], mv[:, 1:2], AF.Sqrt, bias=epsb[:])
            r0 = smallp.tile([128, 1], F32, tag="r0")
            nc.vector.reciprocal(r0[:], y0[:])
            vr = smallp.tile([128, 1], F32, tag="vr")
            nc.vector.tensor_scalar(vr[:], mv[:, 1:2], 1e-5, None, ALU.add)
            t1 = smallp.tile([128, 1], F32, tag="t1")
            nc.vector.tensor_tensor(out=t1[:], in0=vr[:], in1=r0[:], op=ALU.mult)
            y1 = smallp.tile([128, 1], F32, tag="y1")
            nc.vector.tensor_tensor(out=y1[:], in0=y0[:], in1=t1[:], op=ALU.add)
            sV = smallp.tile([128, 1], F32, tag="sV")
            nc.vector.reciprocal(sV[:], y1[:])
            nc.vector.tensor_scalar(sV[:], sV[:], 2.0, None, ALU.mult)
            nb = smallp.tile([128, 1], F32, tag="nb")
            nc.vector.tensor_tensor(out=nb[:], in0=mv[:, 0:1], in1=sV[:],
                                    op=ALU.mult)
            nc.vector.tensor_scalar(nb[:], nb[:], -1.0, None, ALU.mult)

            yt = ypool.tile([128, c.ZD], F32, tag="yt")
            nc.scalar.activation(yt[:], zps[:], AF.Relu, bias=nb[:], scale=sV[:])
            # dot with w2 on gpsimd (frees VE); accum into upre column t
            dscr = ypool.tile([128, c.ZD], F32, tag="dscr")
            nc.vector.scalar_tensor_tensor(out=dscr[:], in0=yt[:], scalar=1.0,
                                           in1=w2rep[:], op0=ALU.mult,
                                           op1=ALU.mult,
                                           accum_out=upre[:, t:t + 1])
        main_ctx.close()

        # ---- scores to DRAM (node-linear) + AllGather ----
        tailps = ctx.enter_context(tc.tile_pool(name="tailps", bufs=1, space="PSUM"))
        tailsb = ctx.enter_context(tc.tile_pool(name="tailsb", bufs=1))
        uT_ps = tailps.tile([128, 128], F32)
        assert c.T <= 128
        nc.tensor.transpose(uT_ps[:c.T, :128], upre[:, :c.T], ident[:])
        uT = tailsb.tile([c.T, 128], F32)
        nc.scalar.copy(uT[:], uT_ps[:c.T, :128])
        u_sh = dram.tile([c.S], F32)
        nc.sync.dma_start(u_sh.rearrange("(t p) -> t p", p=128), uT[:])
        u_full = dram.tile([c.NTOT], F32)
        nc.gpsimd.collective_compute(
            "AllGather", ALU.bypass, replica_groups=[list(range(c.NC))],
            ins=[u_sh.opt()], outs=[u_full.opt()])

        # ---- gather padded per-graph rows, mask, bisect thresholds ----
        gstarts = _HOST_GSTARTS  # static, global (same for all cores)
        bis = []
        for gc in range(c.GC):
            bt = tailsb.tile([128, c.P], F32, tag=f"bis{gc}")
            nc.vector.memset(bt[:], -1e30)
            bis.append(bt)
        for g in range(c.NG):
            gc, r = divmod(g, 128)
            gs = int(gstarts[g])
            nc.sync.dma_start(bis[gc][r:r + 1, 0:c.P], u_full[gs:gs + c.P])
        selt = []
        for gc in range(c.GC):
            st_ = tailsb.tile([128, c.P], I32, tag=f"sel{gc}")
            nc.sync.dma_start(st_[:], i_sel[gc])
            selt.append(st_)
        # bisv = where(sel, bis, -1e30)
        bisv = []
        for gc in range(c.GC):
            bv = tailsb.tile([128, c.P], F32, tag=f"bisv{gc}")
            nc.vector.memset(bv[:], -1e30)
            nc.vector.copy_predicated(bv[:], selt[gc][:], bis[gc][:])
            bisv.append(bv)

        kv = []
        for gc in range(c.GC):
            kt = tailsb.tile([128, 1], F32, tag=f"kv{gc}")
            nc.sync.dma_start(kt[:], i_k[gc])
            kv.append(kt)

        thr = []   # converging lower bounds (end inside the top-k gap)
        for gc in range(c.GC):
            lo_t = tailsb.tile([128, 1], F32, tag=f"lo{gc}")
            nc.vector.memset(lo_t[:], c.BLO)
            hi_t = tailsb.tile([128, 1], F32, tag=f"hi{gc}")
            nc.vector.memset(hi_t[:], c.BHI)
            mid = tailsb.tile([128, 1], F32, tag=f"mid{gc}")
            cmps = tailsb.tile([128, c.P], F32, tag=f"cmps{gc}")
            cnt = tailsb.tile([128, 1], F32, tag=f"cnt{gc}")
            selp = tailsb.tile([128, 1], F32, tag=f"selp{gc}")
            dd = tailsb.tile([128, 1], F32, tag=f"dd{gc}")
            for it in range(c.NITER):
                nc.vector.tensor_tensor(out=mid[:], in0=lo_t[:], in1=hi_t[:],
                                        op=ALU.add)
                nc.vector.tensor_scalar(mid[:], mid[:], 0.5, None, ALU.mult)
                nc.vector.tensor_scalar(cmps[:], bisv[gc][:], mid[:], None,
                                        ALU.is_gt, ALU.add, accum_out=cnt[:])
                nc.vector.tensor_tensor(out=selp[:], in0=cnt[:], in1=kv[gc][:],
                                        op=ALU.is_ge)
                nc.vector.tensor_tensor(out=dd[:], in0=mid[:], in1=lo_t[:],
                                        op=ALU.subtract)
                nc.vector.scalar_tensor_tensor(out=lo_t[:], in0=dd[:],
                                               scalar=selp[:], in1=lo_t[:],
                                               op0=ALU.mult, op1=ALU.add)
                nc.vector.tensor_tensor(out=dd[:], in0=hi_t[:], in1=mid[:],
                                        op=ALU.subtract)
                nc.vector.scalar_tensor_tensor(out=dd[:], in0=dd[:],
                                               scalar=selp[:], in1=mid[:],
                                               op0=ALU.mult, op1=ALU.add)
                nc.vector.tensor_copy(hi_t[:], dd[:])
            thr.append(lo_t)

        # ---- per-node threshold field tau_L[p, t] via staircase matmuls ----
        # thr columns -> one row [1, NG] via PE transpose; delta = thr - prev
        NGP = c.GC * 128
        thr_rowps = tailps.tile([128, 128], F32, tag="thr_rowps")
        thr_row = tailsb.tile([1, NGP], F32)
        for gc in range(c.GC):
            nc.tensor.transpose(thr_rowps[0:1, 0:128], thr[gc][:, 0:1], ident[:])
            nc.scalar.copy(thr_row[0:1, 128 * gc:128 * (gc + 1)],
                           thr_rowps[0:1, 0:128])
        prev = tailsb.tile([1, NGP], F32)
        nc.vector.memset(prev[:], 0.0)
        nc.vector.tensor_copy(prev[0:1, 1:NGP], thr_row[0:1, 0:NGP - 1])
        drow = tailsb.tile([1, NGP], F32)
        nc.vector.tensor_tensor(out=drow[:], in0=thr_row[:], in1=prev[:],
                                op=ALU.subtract)
        # back to columns [128, 1] per chunk, then replicate to [128, 128]
        dcolps = tailps.tile([128, 128], F32, tag="dcolps")
        taups = tailps.tile([128, c.T], F32, tag="taups")
        first = True
        for gc in range(c.GC):
            nc.tensor.transpose(dcolps[0:128, 0:1],
                                drow[0:1, 128 * gc:128 * (gc + 1)],
                                ident[0:1, 0:1])
            dcol = tailsb.tile([128, 1], F32, tag=f"dcol{gc}")
            nc.scalar.copy(dcol[:], dcolps[0:128, 0:1])
            drep = tailsb.tile([128, 128], F32, tag=f"drep{gc}")
            nc.vector.tensor_copy(drep[:], dcol[:].broadcast_to([128, 128]))
            r1t = tailsb.tile([128, c.T], F32, tag=f"r1t{gc}")
            nc.sync.dma_start(r1t[:], i_r1[gc])
            r2t = tailsb.tile([128, c.T], F32, tag=f"r2t{gc}")
            nc.sync.dma_start(r2t[:], i_r2[gc])
            pmt = tailsb.tile([128, 128], F32, tag=f"pmt{gc}")
            nc.sync.dma_start(pmt[:], i_pm[gc])
            lhs2 = tailsb.tile([128, 128], F32, tag=f"lhs2{gc}")
            nc.vector.tensor_tensor(out=lhs2[:], in0=drep[:], in1=pmt[:],
                                    op=ALU.mult)
            nc.tensor.matmul(taups[:, 0:c.T], drep[:], r1t[:],
                             start=first, stop=False)
            first = False
            nc.tensor.matmul(taups[:, 0:c.T], lhs2[:], r2t[:],
                             start=False, stop=(gc == c.GC - 1))
        tau = tailsb.tile([128, c.T], F32)
        nc.scalar.copy(tau[:], taups[:, 0:c.T])

        # ---- masks, logits, a/b fields ----
        mL = tailsb.tile([128, c.T], F32)
        nc.vector.tensor_tensor(out=mL[:], in0=upre[:, :c.T], in1=tau[:],
                                op=ALU.is_gt)
        lsig = tailsb.tile([128, c.T], F32)
        nc.scalar.activation(lsig[:], upre[:, :c.T], AF.Sigmoid, bias=b2b[:])
        aL = tailsb.tile([128, c.T], F32)
        nc.vector.tensor_tensor(out=aL[:], in0=lsig[:], in1=mL[:], op=ALU.mult)
        bL = tailsb.tile([128, c.T], F32)
        nc.vector.tensor_scalar(bL[:], mL[:], -1.0, 1.0, ALU.mult, ALU.add)

        lsigT_ps = tailps.tile([128, 128], F32, tag="lsigT_ps")
        nc.tensor.transpose(lsigT_ps[:c.T, :128], lsig[:, :c.T], ident[:])
        lsigT = tailsb.tile([c.T, 128], F32)
        nc.scalar.copy(lsigT[:], lsigT_ps[:c.T, :128])
        nc.sync.dma_start(o_lg.rearrange("(t p) -> t p", p=128), lsigT[:])

        # ---- x_new apply ----
        XW_ = XW
        xpool = ctx.enter_context(tc.tile_pool(name="xp", bufs=3))
        x_r = i_x.rearrange("(q j p) f -> q p j f", p=128, j=XW_)
        n_r = i_n.rearrange("(q j p) f -> q p j f", p=128, j=XW_)
        o_r = o_xn.rearrange("(q j p) f -> q p j f", p=128, j=XW_)
        for q in range(c.T // XW_):
            xt = xpool.tile([128, XW_, c.XD], F32, tag="xt")
            nc.sync.dma_start(xt[:], x_r[q])
            nt = xpool.tile([128, XW_, c.XD], F32, tag="nt")
            nc.sync.dma_start(nt[:], n_r[q])
            t2_ = xpool.tile([128, XW_, c.XD], F32, tag="t2_")
            ot = xpool.tile([128, XW_, c.XD], F32, tag="ot")
            for j in range(XW_):
                tt = XW_ * q + j
                nc.scalar.activation(t2_[:, j, :], nt[:, j, :], AF.Identity,
                                     scale=bL[:, tt:tt + 1])
                nc.vector.scalar_tensor_tensor(out=ot[:, j, :], in0=xt[:, j, :],
                                               scalar=aL[:, tt:tt + 1],
                                               in1=t2_[:, j, :],
                                               op0=ALU.mult, op1=ALU.add)
            nc.sync.dma_start(o_r[q], ot[:])

    return nc


_HOST_GSTARTS = None  # set before build_program


def _host_plan(batch: np.ndarray, cfg: Cfg):
    """Compute shard/graph bookkeeping from the (sorted) batch vector."""
    c = cfg
    sizes = np.bincount(batch, minlength=c.NG).astype(np.int64)
    starts = np.concatenate([[0], np.cumsum(sizes)])
    k = np.ceil(0.9 * sizes.astype(np.float32)).astype(np.int64)
    gstarts = np.minimum(starts[:c.NG], c.NTOT - c.P)
    shift = starts[:c.NG] - gstarts
    # global sel masks [GC, 128, P] and k vectors [GC, 128, 1]
    sel = np.zeros((c.GC, 128, c.P), np.int32)
    kvec = np.full((c.GC, 128, 1), 1e9, np.float32)
    j = np.arange(c.P)
    for g in range(c.NG):
        gc, r = divmod(g, 128)
        sel[gc, r] = ((j >= shift[g]) & (j < shift[g] + sizes[g])).astype(np.int32)
        kvec[gc, r, 0] = float(k[g])
    # per-core staircase masks
    r1m = np.zeros((c.NC, c.GC, 128, c.T), np.float32)
    r2m = np.zeros((c.NC, c.GC, 128, c.T), np.float32)
    pmm = np.zeros((c.NC, c.GC, 128, 128), np.float32)
    for cc in range(c.NC):
        base = cc * c.S
        for g in range(c.NG):
            gc, r = divmod(g, 128)
            srel = starts[g] - base
            if srel <= 0:
                r1m[cc, gc, r, :] = 1.0
            elif srel >= c.S:
                pass
            else:
                Tg, rg = divmod(int(srel), 128)
                if Tg + 1 < c.T:
                    r1m[cc, gc, r, Tg + 1:] = 1.0
                r2m[cc, gc, r, Tg] = 1.0
                pmm[cc, gc, r, rg:] = 1.0
    return dict(sizes=sizes, starts=starts, k=k, gstarts=gstarts, sel=sel,
                kvec=kvec, r1m=r1m, r2m=r2m, pmm=pmm)


def run(inputs: dict, cfg: Cfg | None = None, sim: bool = False,
        trace: bool = False):
    global _HOST_GSTARTS
    h = np.asarray(inputs["h"], np.float32)
    x = np.asarray(inputs["x"], np.float32)
    noise = np.asarray(inputs["noise"], np.float32)
    W1 = np.asarray(inputs["W1"], np.float32)
    b1 = np.asarray(inputs["b1"], np.float32)
    gamma = np.asarray(inputs["gamma"], np.float32)
    beta = np.asarray(inputs["beta"], np.float32)
    W2 = np.asarray(inputs["W2"], np.float32)
    b2 = np.asarray(inputs["b2"], np.float32)
    batch = np.asarray(inputs["batch"], np.int32)
    edge_index = np.asarray(inputs["edge_index"], np.int32)

    assert not np.any(b1), "b1 != 0 unsupported by this kernel build"
    assert not np.any(beta), "beta != 0 unsupported"
    assert np.all(gamma > 0), "gamma <= 0 unsupported"

    n_nodes = h.shape[0]
    if cfg is None:
        cfg = Cfg()
        ng = int(batch.max()) + 1 if batch.size else 1
        maxg = int(np.bincount(batch).max())
        P = (maxg + 15) // 16 * 16
        cfg = Cfg(P=max(P, 64))
    c = cfg
    assert n_nodes == c.NTOT

    plan = _host_plan(batch, c)
    _HOST_GSTARTS = plan["gstarts"]

    w2g = (gamma * W2[:, 0]).astype(np.float32)
    w2rep = np.repeat(w2g[None, :], 128, 0)

    in_maps = []
    for cc in range(c.NC):
        sl = slice(cc * c.S, (cc + 1) * c.S)
        in_maps.append({
            "h_sh": h[sl], "x_sh": x[sl], "n_sh": noise[sl],
            "w1": W1, "w2rep": w2rep,
            "sel": plan["sel"], "kvec": plan["kvec"],
            "r1m": plan["r1m"][cc], "r2m": plan["r2m"][cc],
            "pmm": plan["pmm"][cc],
        })

    nc = build_program(c, float(b2[0]) if b2.size else 0.0)
    nc.finalize()

    if sim:
        from concourse import bass_interp
        msim = bass_interp.MultiCoreSim(nc, c.NC)
        for cc in range(c.NC):
            for k_, v in in_maps[cc].items():
                msim.cores[cc].tensor(k_)[:] = v
        msim.simulate()
        results = [{"logits_sh": msim.cores[cc].tensor("logits_sh").copy(),
                    "xnew_sh": msim.cores[cc].tensor("xnew_sh").copy()}
                   for cc in range(c.NC)]
        exec_ns = None
    else:
        import time as _time
        t0 = _time.time()
        import os
        tdir = os.environ.get("KERNEL_TRACE_DIR") if trace else None
        if tdir:
            os.makedirs(tdir, exist_ok=True)
        r = run_bass_kernel_spmd(nc, in_maps, list(range(c.NC)), trace=trace,
                                 tmpdir=tdir)
        wall1 = _time.time() - t0
        results = r.results
        exec_ns = r.exec_time_ns
        if exec_ns is None:
            # no NTFF hook in this container: wall-time a second dispatch
            t0 = _time.time()
            r2 = run_bass_kernel_spmd(nc, in_maps, list(range(c.NC)), trace=False)
            wall2 = _time.time() - t0
            print(f"[kernel] spmd wall: first {wall1:.2f}s, second {wall2:.2f}s")

    logits = np.concatenate([np.asarray(results[cc]["logits_sh"])
                             for cc in range(c.NC)])
    x_new = np.concatenate([np.asarray(results[cc]["xnew_sh"])
                            for cc in range(c.NC)])

    edge_mask = np.zeros((n_nodes,), bool)
    edge_mask[edge_index.reshape(-1)] = True
    return (x_new, logits, edge_mask), exec_ns


def kernel(**inputs):
    (x_new, logits, edge_mask), _ = run(inputs)
    return x_new, logits, edge_mask


# revision 24
# speedup vs baseline: 1.0447x; 1.0447x over previous
"""Trainium2 Bass kernel for nn_Attr_sampler (topk_masking).

Pipeline per node: z = h@W1 (+b1); LayerNorm; ReLU; u = y@(gamma*W2); per-graph
top-k on u (monotone in sigmoid(u)); logits = sigmoid(u+b2);
x_new = mask ? x*logits : noise; edge_mask = nodes touched by edge_index.

Strategy:
- 8-way data parallel over equal 16384-node shards (128 tiles of 128 nodes).
- Matmul1 in three full-rate float32r (FP22) passes via exact hi/lo splitting:
  z = hi@Whi + lo@Whi + hi@Wlo, accuracy ~ fp32 (verified 4e-7 on HW).
- Pre-sigmoid scores AllGathered; every core bisects per-graph thresholds for
  all 256 graphs on [128, P] padded rows (static program, no dynamic offsets).
- Per-node threshold field built from host-supplied interval masks with two
  accumulated PE matmuls (telescoped staircase over sorted graph ids).
- x_new applied as x*a + noise*b with a = sigmoid(u)*mask, b = 1-mask.
- edge_mask is an independent int-only side output computed on host.
"""
import math
from contextlib import ExitStack
from dataclasses import dataclass

import numpy as np

import concourse.bass as bass
import concourse.bacc as bacc
import concourse.mybir as mybir
import concourse.tile as tile
from concourse import masks as _cmasks
from concourse.bass_utils import run_bass_kernel_spmd

F32 = mybir.dt.float32
F32R = mybir.dt.float32r
I32 = mybir.dt.int32
AF = mybir.ActivationFunctionType
ALU = mybir.AluOpType


@dataclass(frozen=True)
class Cfg:
    NC: int = 8          # cores
    S: int = 16384       # nodes per core
    HID: int = 512       # h feature dim
    ZD: int = 1024       # hidden dim (2*HID)
    XD: int = 256        # x feature dim
    NG: int = 256        # graphs
    P: int = 592         # padded slots per graph row
    NITER: int = 26      # bisection iterations
    BLO: float = -64.0   # bisection bounds for pre-sigmoid scores
    BHI: float = 64.0

    @property
    def T(self):         # node tiles per core
        return self.S // 128

    @property
    def GC(self):        # graph chunks of 128
        return (self.NG + 127) // 128

    @property
    def NTOT(self):
        return self.NC * self.S


def build_program(cfg: Cfg, b2: float):
    """One SPMD program; all per-core variation comes via input tensors."""
    c = cfg
    KCH = c.HID // 128          # contraction chunks (4)
    NH = c.ZD // 512            # 512-wide feature halves (2)
    XW = 8                      # node tiles per x-apply iteration
    nc = bacc.Bacc("TRN2", target_bir_lowering=False, debug=False,
                   num_devices=cfg.NC)

    i_h = nc.declare_dram_parameter("h_sh", [c.S, c.HID], F32, isOutput=False)
    i_x = nc.declare_dram_parameter("x_sh", [c.S, c.XD], F32, isOutput=False)
    i_n = nc.declare_dram_parameter("n_sh", [c.S, c.XD], F32, isOutput=False)
    i_w1 = nc.declare_dram_parameter("w1", [c.HID, c.ZD], F32, isOutput=False)
    i_w2 = nc.declare_dram_parameter("w2rep", [128, c.ZD], F32, isOutput=False)
    i_sel = nc.declare_dram_parameter("sel", [c.GC, 128, c.P], I32, isOutput=False)
    i_k = nc.declare_dram_parameter("kvec", [c.GC, 128, 1], F32, isOutput=False)
    i_r1 = nc.declare_dram_parameter("r1m", [c.GC, 128, c.T], F32, isOutput=False)
    i_r2 = nc.declare_dram_parameter("r2m", [c.GC, 128, c.T], F32, isOutput=False)
    i_pm = nc.declare_dram_parameter("pmm", [c.GC, 128, 128], F32, isOutput=False)
    o_lg = nc.declare_dram_parameter("logits_sh", [c.S], F32, isOutput=True)
    o_xn = nc.declare_dram_parameter("xnew_sh", [c.S, c.XD], F32, isOutput=True)

    with tile.TileContext(nc) as tc, ExitStack() as ctx:
        consts = ctx.enter_context(tc.tile_pool(name="consts", bufs=1))
        dram = ctx.enter_context(tc.tile_pool(name="dram", bufs=1, space="DRAM"))

        ident = consts.tile([128, 128], F32)
        io_r = consts.tile([128, 128], I32)
        nc.gpsimd.iota(io_r[:], pattern=[[1, 128]], base=0, channel_multiplier=0)
        io_p = consts.tile([128, 1], I32)
        nc.gpsimd.iota(io_p[:], pattern=[[1, 1]], base=0, channel_multiplier=1)
        io_pf = consts.tile([128, 1], F32)
        nc.vector.tensor_copy(io_pf[:], io_p[:])
        io_rf = consts.tile([128, 128], F32)
        nc.vector.tensor_copy(io_rf[:], io_r[:])
        nc.vector.tensor_scalar(ident[:], io_rf[:], io_pf[:], None, ALU.is_equal)
        identr = consts.tile([128, 128], F32R)
        nc.vector.tensor_copy(identr[:], ident[:])
        epsb = consts.tile([128, 1], F32)
        nc.vector.memset(epsb[:], 1e-5)
        b2b = consts.tile([128, 1], F32)
        nc.vector.memset(b2b[:], float(b2))

        # ---- stage + split weights (one-time; staging freed after) ----
        w1hi = consts.tile([128, KCH, c.ZD], F32R)
        w1lo = consts.tile([128, KCH, c.ZD], F32R)
        wstage_ctx = ExitStack()
        wstage = wstage_ctx.enter_context(tc.tile_pool(name="wstage", bufs=1))
        w1f = wstage.tile([128, KCH, c.ZD], F32)
        nc.sync.dma_start(w1f[:], i_w1.rearrange("(k p) m -> p k m", p=128))
        nc.vector.tensor_copy(w1hi[:], w1f[:])
        nc.vector.tensor_tensor(out=w1lo[:], in0=w1f[:],
                                in1=w1hi.bitcast(F32)[:], op=ALU.subtract)
        wstage_ctx.close()
        w2rep = consts.tile([128, c.ZD], F32)
        nc.sync.dma_start(w2rep[:], i_w2[:])

        upre = consts.tile([128, c.T], F32)   # pre-sigmoid scores, [p, t]

        # pools used after the main loop, hoisted so their SBUF ranges are
        # disjoint from the main pools (no release-gating; enables prefetch)
        tailsb = ctx.enter_context(tc.tile_pool(name="tailsb", bufs=1))
        xinp = ctx.enter_context(tc.tile_pool(name="xin", bufs=3))
        xpool = ctx.enter_context(tc.tile_pool(name="xp", bufs=2))

        # ---- main loop over node tiles (own stack so PSUM frees for tail) ----
        main_ctx = ExitStack()
        hpool = main_ctx.enter_context(tc.tile_pool(name="hp", bufs=3))
        spool = main_ctx.enter_context(tc.tile_pool(name="sp", bufs=2))
        tpool = main_ctx.enter_context(tc.tile_pool(name="tp", bufs=2, space="PSUM"))
        zpool = main_ctx.enter_context(tc.tile_pool(name="zp", bufs=2, space="PSUM"))
        ypool = main_ctx.enter_context(tc.tile_pool(name="yp", bufs=2))
        scrpool = main_ctx.enter_context(tc.tile_pool(name="scr", bufs=1))
        smallp = main_ctx.enter_context(tc.tile_pool(name="smp", bufs=3))

        h_r = i_h.rearrange("(t p) f -> t p f", p=128)
        for t in range(c.T):
            ht = hpool.tile([128, c.HID], F32, tag="ht")
            nc.sync.dma_start(ht[:], h_r[t])
            hhi = spool.tile([128, c.HID], F32R, tag="hhi")
            nc.vector.tensor_copy(hhi[:], ht[:])
            hlo = spool.tile([128, c.HID], F32R, tag="hlo")
            nc.vector.tensor_tensor(out=hlo[:], in0=ht[:],
                                    in1=hhi.bitcast(F32)[:], op=ALU.subtract)
            # transpose both (PE), PSUM -> SBUF copies on ACT
            tps = tpool.tile([128, 2 * c.HID], F32R, tag="tps")
            for k in range(KCH):
                nc.tensor.transpose(tps[:, 128 * k:128 * (k + 1)],
                                    hhi[:, 128 * k:128 * (k + 1)],
                                    identr[:])
            for k in range(KCH):
                nc.tensor.transpose(tps[:, c.HID + 128 * k:c.HID + 128 * (k + 1)],
                                    hlo[:, 128 * k:128 * (k + 1)],
                                    identr[:])
            hiT = spool.tile([128, c.HID], F32R, tag="hiT")
            nc.scalar.copy(hiT[:], tps[:, 0:c.HID])
            loT = spool.tile([128, c.HID], F32R, tag="loT")
            nc.scalar.copy(loT[:], tps[:, c.HID:2 * c.HID])

            zps = zpool.tile([128, c.ZD], F32, tag="zps")
            for hh in range(NH):
                i = 0
                for k in range(KCH):
                    for (a, b) in ((hiT, w1hi), (loT, w1hi), (hiT, w1lo)):
                        nc.tensor.matmul(
                            zps[:, 512 * hh:512 * (hh + 1)],
                            a[:, 128 * k:128 * (k + 1)],
                            b[:, k, 512 * hh:512 * (hh + 1)],
                            start=(i == 0), stop=(i == 3 * KCH - 1))
                        i += 1

            stats = smallp.tile([128, NH, 6], F32, tag="stats")
            for hh in range(NH):
                nc.vector.bn_stats(stats[:, hh, :], zps[:, 512 * hh:512 * (hh + 1)])
            mv = smallp.tile([128, 2], F32, tag="mv")
            nc.vector.bn_aggr(mv[:], stats[:])
            # s = 1/sqrt(var+eps) with one Newton refine of the sqrt
            y0 = smallp.tile([128, 1], F32, tag="y0")
            nc.scalar.activation(y0[:], mv[:, 1:2], AF.Sqrt, bias=epsb[:])
            r0 = smallp.tile([128, 1], F32, tag="r0")
            nc.vector.reciprocal(r0[:], y0[:])
            vr = smallp.tile([128, 1], F32, tag="vr")
            nc.vector.tensor_scalar(vr[:], mv[:, 1:2], 1e-5, None, ALU.add)
            t1 = smallp.tile([128, 1], F32, tag="t1")
            nc.vector.tensor_tensor(out=t1[:], in0=vr[:], in1=r0[:], op=ALU.mult)
            y1 = smallp.tile([128, 1], F32, tag="y1")
            nc.vector.tensor_tensor(out=y1[:], in0=y0[:], in1=t1[:], op=ALU.add)
            sV = smallp.tile([128, 1], F32, tag="sV")
            nc.vector.reciprocal(sV[:], y1[:])
            nc.vector.tensor_scalar(sV[:], sV[:], 2.0, None, ALU.mult)
            nb = smallp.tile([128, 1], F32, tag="nb")
            nc.vector.tensor_tensor(out=nb[:], in0=mv[:, 0:1], in1=sV[:],
                                    op=ALU.mult)
            nc.vector.tensor_scalar(nb[:], nb[:], -1.0, None, ALU.mult)

            yt = ypool.tile([128, c.ZD], F32, tag="yt")
            nc.scalar.activation(yt[:], zps[:], AF.Relu, bias=nb[:], scale=sV[:])
            # dot with w2 on gpsimd (frees VE); accum into upre column t
            dscr = scrpool.tile([128, c.ZD], F32, tag="dscr")
            nc.vector.scalar_tensor_tensor(out=dscr[:], in0=yt[:], scalar=1.0,
                                           in1=w2rep[:], op0=ALU.mult,
                                           op1=ALU.mult,
                                           accum_out=upre[:, t:t + 1])
        main_ctx.close()

        # ---- scores to DRAM (node-linear) + AllGather ----
        tailps = ctx.enter_context(tc.tile_pool(name="tailps", bufs=1, space="PSUM"))
        uT_ps = tailps.tile([128, 128], F32)
        assert c.T <= 128
        nc.tensor.transpose(uT_ps[:c.T, :128], upre[:, :c.T], ident[:])
        uT = tailsb.tile([c.T, 128], F32)
        nc.scalar.copy(uT[:], uT_ps[:c.T, :128])
        u_sh = dram.tile([c.S], F32)
        nc.sync.dma_start(u_sh.rearrange("(t p) -> t p", p=128), uT[:])
        u_full = dram.tile([c.NTOT], F32)
        nc.gpsimd.collective_compute(
            "AllGather", ALU.bypass, replica_groups=[list(range(c.NC))],
            ins=[u_sh.opt()], outs=[u_full.opt()])

        # ---- gather padded per-graph rows, mask, bisect thresholds ----
        gstarts = _HOST_GSTARTS  # static, global (same for all cores)
        bis = []
        for gc in range(c.GC):
            bt = tailsb.tile([128, c.P], F32, tag=f"bis{gc}")
            nc.vector.memset(bt[:], -1e30)
            bis.append(bt)
        _gather_engines = [nc.sync, nc.scalar, nc.gpsimd]
        for g in range(c.NG):
            gc, r = divmod(g, 128)
            gs = int(gstarts[g])
            eng = _gather_engines[g % 3]
            eng.dma_start(bis[gc][r:r + 1, 0:c.P], u_full[gs:gs + c.P])
        selt = []
        for gc in range(c.GC):
            st_ = tailsb.tile([128, c.P], I32, tag=f"sel{gc}")
            nc.gpsimd.dma_start(st_[:], i_sel[gc])
            selt.append(st_)
        # bisv = where(sel, bis, -1e30)
        bisv = []
        for gc in range(c.GC):
            bv = tailsb.tile([128, c.P], F32, tag=f"bisv{gc}")
            nc.vector.memset(bv[:], -1e30)
            nc.vector.copy_predicated(bv[:], selt[gc][:], bis[gc][:])
            bisv.append(bv)

        kv = []
        for gc in range(c.GC):
            kt = tailsb.tile([128, 1], F32, tag=f"kv{gc}")
            nc.gpsimd.dma_start(kt[:], i_k[gc])
            kv.append(kt)

        thr = []   # converging lower bounds (end inside the top-k gap)
        for gc in range(c.GC):
            lo_t = tailsb.tile([128, 1], F32, tag=f"lo{gc}")
            nc.vector.memset(lo_t[:], c.BLO)
            hi_t = tailsb.tile([128, 1], F32, tag=f"hi{gc}")
            nc.vector.memset(hi_t[:], c.BHI)
            mid = tailsb.tile([128, 1], F32, tag=f"mid{gc}")
            cmps = tailsb.tile([128, c.P], F32, tag=f"cmps{gc}")
            cnt = tailsb.tile([128, 1], F32, tag=f"cnt{gc}")
            selp = tailsb.tile([128, 1], F32, tag=f"selp{gc}")
            dd = tailsb.tile([128, 1], F32, tag=f"dd{gc}")
            for it in range(c.NITER):
                nc.vector.tensor_tensor(out=mid[:], in0=lo_t[:], in1=hi_t[:],
                                        op=ALU.add)
                nc.vector.tensor_scalar(mid[:], mid[:], 0.5, None, ALU.mult)
                nc.vector.tensor_scalar(cmps[:], bisv[gc][:], mid[:], None,
                                        ALU.is_gt, ALU.add, accum_out=cnt[:])
                nc.vector.tensor_tensor(out=selp[:], in0=cnt[:], in1=kv[gc][:],
                                        op=ALU.is_ge)
                nc.vector.tensor_tensor(out=dd[:], in0=mid[:], in1=lo_t[:],
                                        op=ALU.subtract)
                nc.vector.scalar_tensor_tensor(out=lo_t[:], in0=dd[:],
                                               scalar=selp[:], in1=lo_t[:],
                                               op0=ALU.mult, op1=ALU.add)
                nc.vector.tensor_tensor(out=dd[:], in0=hi_t[:], in1=mid[:],
                                        op=ALU.subtract)
                nc.vector.scalar_tensor_tensor(out=dd[:], in0=dd[:],
                                               scalar=selp[:], in1=mid[:],
                                               op0=ALU.mult, op1=ALU.add)
                nc.vector.tensor_copy(hi_t[:], dd[:])
            thr.append(lo_t)

        # ---- per-node threshold field tau_L[p, t] via staircase matmuls ----
        # thr columns -> one row [1, NG] via PE transpose; delta = thr - prev
        NGP = c.GC * 128
        thr_rowps = tailps.tile([128, 128], F32, tag="thr_rowps")
        thr_row = tailsb.tile([1, NGP], F32)
        for gc in range(c.GC):
            nc.tensor.transpose(thr_rowps[0:1, 0:128], thr[gc][:, 0:1], ident[:])
            nc.scalar.copy(thr_row[0:1, 128 * gc:128 * (gc + 1)],
                           thr_rowps[0:1, 0:128])
        prev = tailsb.tile([1, NGP], F32)
        nc.vector.memset(prev[:], 0.0)
        nc.vector.tensor_copy(prev[0:1, 1:NGP], thr_row[0:1, 0:NGP - 1])
        drow = tailsb.tile([1, NGP], F32)
        nc.vector.tensor_tensor(out=drow[:], in0=thr_row[:], in1=prev[:],
                                op=ALU.subtract)
        # back to columns [128, 1] per chunk, then replicate to [128, 128]
        dcolps = tailps.tile([128, 128], F32, tag="dcolps")
        taups = tailps.tile([128, c.T], F32, tag="taups")
        first = True
        for gc in range(c.GC):
            nc.tensor.transpose(dcolps[0:128, 0:1],
                                drow[0:1, 128 * gc:128 * (gc + 1)],
                                ident[0:1, 0:1])
            dcol = tailsb.tile([128, 1], F32, tag=f"dcol{gc}")
            nc.scalar.copy(dcol[:], dcolps[0:128, 0:1])
            drep = tailsb.tile([128, 128], F32, tag=f"drep{gc}")
            nc.vector.tensor_copy(drep[:], dcol[:].broadcast_to([128, 128]))
            r1t = tailsb.tile([128, c.T], F32, tag=f"r1t{gc}")
            nc.gpsimd.dma_start(r1t[:], i_r1[gc])
            r2t = tailsb.tile([128, c.T], F32, tag=f"r2t{gc}")
            nc.gpsimd.dma_start(r2t[:], i_r2[gc])
            pmt = tailsb.tile([128, 128], F32, tag=f"pmt{gc}")
            nc.gpsimd.dma_start(pmt[:], i_pm[gc])
            lhs2 = tailsb.tile([128, 128], F32, tag=f"lhs2{gc}")
            nc.vector.tensor_tensor(out=lhs2[:], in0=drep[:], in1=pmt[:],
                                    op=ALU.mult)
            nc.tensor.matmul(taups[:, 0:c.T], drep[:], r1t[:],
                             start=first, stop=False)
            first = False
            nc.tensor.matmul(taups[:, 0:c.T], lhs2[:], r2t[:],
                             start=False, stop=(gc == c.GC - 1))
        tau = tailsb.tile([128, c.T], F32)
        nc.scalar.copy(tau[:], taups[:, 0:c.T])

        # ---- masks, logits, a/b fields ----
        mL = tailsb.tile([128, c.T], F32)
        nc.vector.tensor_tensor(out=mL[:], in0=upre[:, :c.T], in1=tau[:],
                                op=ALU.is_gt)
        lsig = tailsb.tile([128, c.T], F32)
        nc.scalar.activation(lsig[:], upre[:, :c.T], AF.Sigmoid, bias=b2b[:])
        aL = tailsb.tile([128, c.T], F32)
        nc.vector.tensor_tensor(out=aL[:], in0=lsig[:], in1=mL[:], op=ALU.mult)
        bL = tailsb.tile([128, c.T], F32)
        nc.vector.tensor_scalar(bL[:], mL[:], -1.0, 1.0, ALU.mult, ALU.add)

        lsigT_ps = tailps.tile([128, 128], F32, tag="lsigT_ps")
        nc.tensor.transpose(lsigT_ps[:c.T, :128], lsig[:, :c.T], ident[:])
        lsigT = tailsb.tile([c.T, 128], F32)
        nc.scalar.copy(lsigT[:], lsigT_ps[:c.T, :128])
        nc.sync.dma_start(o_lg.rearrange("(t p) -> t p", p=128), lsigT[:])

        # ---- x_new apply ----
        XW_ = XW
        x_r = i_x.rearrange("(q j p) f -> q p j f", p=128, j=XW_)
        n_r = i_n.rearrange("(q j p) f -> q p j f", p=128, j=XW_)
        o_r = o_xn.rearrange("(q j p) f -> q p j f", p=128, j=XW_)
        for q in range(c.T // XW_):
            xt = xinp.tile([128, XW_, c.XD], F32, tag="xt")
            nc.sync.dma_start(xt[:], x_r[q])
            nt = xinp.tile([128, XW_, c.XD], F32, tag="nt")
            nc.scalar.dma_start(nt[:], n_r[q])
            t2_ = xpool.tile([128, XW_, c.XD], F32, tag="t2_")
            ot = xpool.tile([128, XW_, c.XD], F32, tag="ot")
            for j in range(XW_):
                tt = XW_ * q + j
                nc.scalar.activation(t2_[:, j, :], nt[:, j, :], AF.Identity,
                                     scale=bL[:, tt:tt + 1])
                nc.vector.scalar_tensor_tensor(out=ot[:, j, :], in0=xt[:, j, :],
                                               scalar=aL[:, tt:tt + 1],
                                               in1=t2_[:, j, :],
                                               op0=ALU.mult, op1=ALU.add)
            nc.scalar.dma_start(o_r[q], ot[:])

    return nc


_HOST_GSTARTS = None  # set before build_program


def _host_plan(batch: np.ndarray, cfg: Cfg):
    """Compute shard/graph bookkeeping from the (sorted) batch vector."""
    c = cfg
    sizes = np.bincount(batch, minlength=c.NG).astype(np.int64)
    starts = np.concatenate([[0], np.cumsum(sizes)])
    k = np.ceil(0.9 * sizes.astype(np.float32)).astype(np.int64)
    gstarts = np.minimum(starts[:c.NG], c.NTOT - c.P)
    shift = starts[:c.NG] - gstarts
    # global sel masks [GC, 128, P] and k vectors [GC, 128, 1]
    sel = np.zeros((c.GC, 128, c.P), np.int32)
    kvec = np.full((c.GC, 128, 1), 1e9, np.float32)
    j = np.arange(c.P)
    for g in range(c.NG):
        gc, r = divmod(g, 128)
        sel[gc, r] = ((j >= shift[g]) & (j < shift[g] + sizes[g])).astype(np.int32)
        kvec[gc, r, 0] = float(k[g])
    # per-core staircase masks
    r1m = np.zeros((c.NC, c.GC, 128, c.T), np.float32)
    r2m = np.zeros((c.NC, c.GC, 128, c.T), np.float32)
    pmm = np.zeros((c.NC, c.GC, 128, 128), np.float32)
    for cc in range(c.NC):
        base = cc * c.S
        for g in range(c.NG):
            gc, r = divmod(g, 128)
            srel = starts[g] - base
            if srel <= 0:
                r1m[cc, gc, r, :] = 1.0
            elif srel >= c.S:
                pass
            else:
                Tg, rg = divmod(int(srel), 128)
                if Tg + 1 < c.T:
                    r1m[cc, gc, r, Tg + 1:] = 1.0
                r2m[cc, gc, r, Tg] = 1.0
                pmm[cc, gc, r, rg:] = 1.0
    return dict(sizes=sizes, starts=starts, k=k, gstarts=gstarts, sel=sel,
                kvec=kvec, r1m=r1m, r2m=r2m, pmm=pmm)


def run(inputs: dict, cfg: Cfg | None = None, sim: bool = False,
        trace: bool = False):
    global _HOST_GSTARTS
    h = np.asarray(inputs["h"], np.float32)
    x = np.asarray(inputs["x"], np.float32)
    noise = np.asarray(inputs["noise"], np.float32)
    W1 = np.asarray(inputs["W1"], np.float32)
    b1 = np.asarray(inputs["b1"], np.float32)
    gamma = np.asarray(inputs["gamma"], np.float32)
    beta = np.asarray(inputs["beta"], np.float32)
    W2 = np.asarray(inputs["W2"], np.float32)
    b2 = np.asarray(inputs["b2"], np.float32)
    batch = np.asarray(inputs["batch"], np.int32)
    edge_index = np.asarray(inputs["edge_index"], np.int32)

    assert not np.any(b1), "b1 != 0 unsupported by this kernel build"
    assert not np.any(beta), "beta != 0 unsupported"
    assert np.all(gamma > 0), "gamma <= 0 unsupported"

    n_nodes = h.shape[0]
    if cfg is None:
        cfg = Cfg()
        ng = int(batch.max()) + 1 if batch.size else 1
        maxg = int(np.bincount(batch).max())
        P = (maxg + 15) // 16 * 16
        cfg = Cfg(P=max(P, 64))
    c = cfg
    assert n_nodes == c.NTOT

    plan = _host_plan(batch, c)
    _HOST_GSTARTS = plan["gstarts"]

    w2g = (gamma * W2[:, 0]).astype(np.float32)
    w2rep = np.repeat(w2g[None, :], 128, 0)

    in_maps = []
    for cc in range(c.NC):
        sl = slice(cc * c.S, (cc + 1) * c.S)
        in_maps.append({
            "h_sh": h[sl], "x_sh": x[sl], "n_sh": noise[sl],
            "w1": W1, "w2rep": w2rep,
            "sel": plan["sel"], "kvec": plan["kvec"],
            "r1m": plan["r1m"][cc], "r2m": plan["r2m"][cc],
            "pmm": plan["pmm"][cc],
        })

    nc = build_program(c, float(b2[0]) if b2.size else 0.0)
    nc.finalize()

    if sim:
        from concourse import bass_interp
        msim = bass_interp.MultiCoreSim(nc, c.NC)
        for cc in range(c.NC):
            for k_, v in in_maps[cc].items():
                msim.cores[cc].tensor(k_)[:] = v
        msim.simulate()
        results = [{"logits_sh": msim.cores[cc].tensor("logits_sh").copy(),
                    "xnew_sh": msim.cores[cc].tensor("xnew_sh").copy()}
                   for cc in range(c.NC)]
        exec_ns = None
    else:
        import time as _time
        t0 = _time.time()
        import os
        tdir = os.environ.get("KERNEL_TRACE_DIR") if trace else None
        if tdir:
            os.makedirs(tdir, exist_ok=True)
        r = run_bass_kernel_spmd(nc, in_maps, list(range(c.NC)), trace=trace,
                                 tmpdir=tdir)
        wall1 = _time.time() - t0
        results = r.results
        exec_ns = r.exec_time_ns
        if exec_ns is None and os.environ.get("KERNEL_TIME_SECOND"):
            t0 = _time.time()
            run_bass_kernel_spmd(nc, in_maps, list(range(c.NC)), trace=False)
            wall2 = _time.time() - t0
            print(f"[kernel] spmd wall: first {wall1:.2f}s, second {wall2:.2f}s")

    logits = np.concatenate([np.asarray(results[cc]["logits_sh"])
                             for cc in range(c.NC)])
    x_new = np.concatenate([np.asarray(results[cc]["xnew_sh"])
                            for cc in range(c.NC)])

    edge_mask = np.zeros((n_nodes,), bool)
    edge_mask[edge_index.reshape(-1)] = True
    return (x_new, logits, edge_mask), exec_ns


def kernel(**inputs):
    (x_new, logits, edge_mask), _ = run(inputs)
    return x_new, logits, edge_mask
